# revision 1
# baseline (speedup 1.0000x reference)
"""Trainium2 Bass kernel for nn_End2EndTongueROI_Dynamic_NMS.

Key algebraic facts used (verified against the reference):
  - Greedy NMS always keeps the top-scored box first and fi=argmax(keep)=0,
    so the whole top-k/NMS tail reduces to argmax(score) over 8400 anchors.
  - score's /max(maskness) normalization and /32 mean are positive scalings
    shared by all anchors -> argmax-invariant -> dropped on device.
  - Both resizes are linear: expressed as matmuls with exact f32 weight
    matrices replicated from jax.image.resize's compute_weight_mat.
  - The final rect is data-dependent but narrow (box w,h ~ U[0,1) in the
    reference's encoding, and the rect is built from *unscaled* xyxy), so
    each core computes its 270-row shard restricted to a dynamic 128-row x
    516-column window that covers the rect. Everything outside the window is
    exactly 0 in the reference output and the PJRT path pre-zeroes/donates
    output buffers (documented contract: "kernels that don't write every
    element rely on that"), so only the window is written. A host-side check
    falls back to exact numpy if the rect ever exceeds the window
    (impossible for in-distribution inputs).
  - The mask pipeline is computed only over the window's dependency cone:
    24 of 160 proto rows (H), a dynamic 28-wide column slice (W), a 2-matmul
    coef matvec, one matmul per resize leg, at exact jax f32 weights.

Sharding: H0=2160 rows split 8 x 270. Score fusion + argmax tail is tiny and
fully replicated per core (no collectives needed).
"""
import numpy as np

import concourse.bacc as bacc
import concourse.bass as bass
import concourse.mybir as mybir
import concourse.tile as tile
from concourse import bass_isa, bass_utils

F32 = mybir.dt.float32
I32 = mybir.dt.int32
U32 = mybir.dt.uint32

N_CORES = 8
H0, W0 = 2160, 3840
IMGSZ = 640
MASK_THR = 0.72
NANCH, NC_COL = 8400, 37
ROWS = H0 // N_CORES          # 270 rows per core
SROWS = 82                    # s640 row window per core
MROWS = 24                    # m160 row window per core (padded)
WWIN = 516                    # output column window (6*86)
SWIN = 88                     # s-column window feeding WWIN
WW160 = 28                    # m160 column window feeding SWIN
RWIN = 128                    # output row window (one partition tile)
NPP = 66                      # anchors per partition (66*128 = 8448 >= 8400)
# sentinel for the argmin-over-winners trick; power of two > NANCH so that
# af - BIG and +BIG round-trip exactly in f32 (af < 2^14, ulp stays <= 1)
BIG = 16384.0


# ---------------------------------------------------------------------------
# host-side resize weights (exact replica of jax.image.resize bilinear)
# ---------------------------------------------------------------------------

def _weight_mat(in_size, out_size):
    dt = np.float32
    scale = dt(out_size / in_size)
    inv_scale = dt(1.0) / scale
    sample_f = (np.arange(out_size, dtype=dt) + dt(0.5)) * inv_scale - dt(0.5)
    x = np.abs(sample_f[None, :] - np.arange(in_size, dtype=dt)[:, None])
    w = np.maximum(dt(0), dt(1) - x).astype(dt)
    tot = w.sum(axis=0, keepdims=True).astype(dt)
    w = np.where(np.abs(tot) > 1000.0 * np.finfo(np.float32).eps,
                 w / np.where(tot != 0, tot, 1), 0).astype(dt)
    ok = (sample_f >= -0.5) & (sample_f <= in_size - 0.5)
    return np.where(ok[None, :], w, 0).astype(dt)


def _host_consts():
    """Constant tensors. Returns (shared, percore_list)."""
    Ah = _weight_mat(160, IMGSZ)      # [160, 640]
    Aw = _weight_mat(160, IMGSZ)      # [160, 640]
    Vh = _weight_mat(IMGSZ, H0)       # [640, 2160]
    Vw = _weight_mat(IMGSZ, W0)       # [640, 3840]

    # vwpad row i+1 = Vw row i (s-col i); zero guard rows at both ends so the
    # dynamic [SWIN, WWIN] slice at row m covers s-cols [m-1, m+86] with the
    # out-of-range ends contributing exactly zero.
    vwpad = np.zeros((642, W0), np.float32)
    vwpad[1:641] = Vw
    # AwT with the same one-column zero guard on both sides: awtp[w, j+1] =
    # Aw[w, j].  The dynamic [WW160, SWIN] slice at (ww, m) then aligns
    # column-for-column with the vwpad slice rows.
    awtp = np.zeros((160, 642), np.float32)
    awtp[:, 1:641] = Aw

    ones1 = np.ones((1, 128), np.float32)
    id1 = np.ones((1, 1), np.float32)
    i128 = np.eye(128, dtype=np.float32)
    xiota = np.broadcast_to(np.arange(WWIN, dtype=np.float32),
                            (128, WWIN)).copy()
    pio1 = np.arange(128, dtype=np.float32).reshape(128, 1).copy()
    pio66 = (np.arange(128, dtype=np.float32) * NPP).reshape(128, 1).copy()

    percore = []
    for c in range(N_CORES):
        r0 = ROWS * c
        vh_sl = Vh[:, r0:r0 + ROWS]
        nz = np.where(vh_sl.any(axis=1))[0]
        ra = min(int(nz.min()), IMGSZ - SROWS)
        r82 = np.ascontiguousarray(vh_sl[ra:ra + SROWS, :])     # [82, 270]

        ah_sl = Ah[:, ra:ra + SROWS]                            # [160, 82]
        nzh = np.where(ah_sl.any(axis=1))[0]
        ha = min(int(nzh.min()), 160 - MROWS)
        ahst = np.ascontiguousarray(ah_sl[ha:ha + MROWS, :])    # [24, 82]

        r0c = np.full((1, 1), float(r0), np.float32)
        percore.append(dict(ra=ra, ha=ha, r82=r82, ahst=ahst, r0c=r0c))

    shared = dict(awtp=awtp, vwpad=vwpad, ones1=ones1, id1=id1, i128=i128,
                  xiota=xiota, pio1=pio1, pio66=pio66)
    return shared, percore


# ---------------------------------------------------------------------------
# device program (identical for all cores; per-core data comes via inputs)
# ---------------------------------------------------------------------------

def _build_nc(stage=99, reps=1, loop_n=0):
    nc = bacc.Bacc("TRN2", target_bir_lowering=False, debug=False,
                   enable_asserts=False, num_devices=N_CORES)

    d = {}
    d["pred"] = nc.dram_tensor("pred", [NANCH, NC_COL], F32, kind="ExternalInput")
    d["xs"] = nc.dram_tensor("xs", [3, ROWS, W0], F32, kind="ExternalInput")
    d["protos"] = nc.dram_tensor("protos", [32, MROWS * 160], F32, kind="ExternalInput")
    d["ahst"] = nc.dram_tensor("ahst", [MROWS, SROWS], F32, kind="ExternalInput")
    d["awtp"] = nc.dram_tensor("awtp", [160, 642], F32, kind="ExternalInput")
    d["r82"] = nc.dram_tensor("r82", [SROWS, ROWS], F32, kind="ExternalInput")
    d["vwpad"] = nc.dram_tensor("vwpad", [642, W0], F32, kind="ExternalInput")
    d["ones1"] = nc.dram_tensor("ones1", [1, 128], F32, kind="ExternalInput")
    d["id1"] = nc.dram_tensor("id1", [1, 1], F32, kind="ExternalInput")
    d["i128"] = nc.dram_tensor("i128", [128, 128], F32, kind="ExternalInput")
    d["xiota"] = nc.dram_tensor("xiota", [128, WWIN], F32, kind="ExternalInput")
    d["pio1"] = nc.dram_tensor("pio1", [128, 1], F32, kind="ExternalInput")
    d["pio66"] = nc.dram_tensor("pio66", [128, 1], F32, kind="ExternalInput")
    d["r0c"] = nc.dram_tensor("r0c", [1, 1], F32, kind="ExternalInput")

    d["out"] = nc.dram_tensor("out", [3, ROWS, W0], F32, kind="ExternalOutput")
    d["meta"] = nc.dram_tensor("meta", [1, 8], F32, kind="ExternalOutput")

    with tile.TileContext(nc) as tc:
        if loop_n:
            with tc.For_i(0, loop_n, 1):
                _program(nc, tc, d, stage, 0)
        else:
            for rep in range(reps):
                _program(nc, tc, d, stage, rep)
    nc.compile()
    return nc


def _program(nc, tc, d, stage=99, rep=0):
    AF = mybir.ActivationFunctionType
    OP = mybir.AluOpType
    AX = mybir.AxisListType
    import contextlib
    ctx = contextlib.ExitStack()

    sb = ctx.enter_context(tc.tile_pool(name="sb", bufs=1))
    ps = ctx.enter_context(tc.tile_pool(name="ps", bufs=2,
                                        space=bass.MemorySpace.PSUM))

    _bias_cache = {}

    def cbias(val):
        if val not in _bias_cache:
            t = sb.tile([128, 1], F32, tag=f"cb{len(_bias_cache)}",
                        name=f"cb{rep}_{len(_bias_cache)}")
            nc.vector.memset(t[:, :], val)
            _bias_cache[val] = t
        return _bias_cache[val]

    def act(out_ap, in_ap, func, bias=0.0, scale=1.0):
        nparts = in_ap.shape[0]
        nc.scalar.activation(out_ap, in_ap, func,
                             bias=cbias(float(bias))[0:nparts, :],
                             scale=scale)

    def ts(out_ap, in_ap, s1, s2, op0, op1=None):
        nc.vector.tensor_scalar(out_ap, in_ap, s1, s2, op0,
                                *([] if op1 is None else [op1]))

    def tt(out_ap, a_ap, b_ap, op):
        nc.vector.tensor_tensor(out_ap, a_ap, b_ap, op)

    def tile1(tag, shape=(128, 1), dtype=F32):
        return sb.tile(list(shape), dtype, tag=tag, name=f"{tag}_{rep}")

    # small consts
    ones1 = tile1("ones1", (1, 128))
    nc.sync.dma_start(ones1[:, :], d["ones1"].ap())
    id1 = tile1("id1", (1, 1))
    nc.sync.dma_start(id1[:, :], d["id1"].ap())
    pio1 = tile1("pio1")
    nc.sync.dma_start(pio1[:, :], d["pio1"].ap())
    pio66 = tile1("pio66")
    nc.sync.dma_start(pio66[:, :], d["pio66"].ap())
    r0c = tile1("r0c", (1, 1))
    nc.sync.dma_start(r0c[:, :], d["r0c"].ap())
    i128 = tile1("i128", (128, 128))
    nc.sync.dma_start(i128[:, :], d["i128"].ap())

    def allmax_col(name, src_col):
        """cross-partition max of [128,1] -> [128,1] broadcast, via PE
        transpose + DVE free-dim reduce + K=1 broadcast matmul (the gpsimd
        partition_all_reduce ucode op is much slower on HW)."""
        pt = ps.tile([1, 128], F32, tag="ps", name=f"pst_{name}_{rep}")
        nc.tensor.transpose(pt[:, :], src_col, i128[:, :])
        m11 = tile1(name + "_m")
        nc.vector.tensor_reduce(m11[0:1, :], pt[:, :], AX.X, OP.max)
        return bcast_col(name, m11[0:1, :])

    def bcast_col(name, src11):
        """[1,1] f32 -> [128,1] via K=1 matmul + copy."""
        p = ps.tile([128, 1], F32, tag="ps", name=f"psb_{name}_{rep}")
        nc.tensor.matmul(p[:, :], ones1[:, :], src11, start=True, stop=True)
        o = tile1(name)
        nc.scalar.copy(o[:, :], p[:, :])
        return o

    # ---------------- stage S: score fusion + argmax ----------------
    # anchor a = p*66 + n; partition 127 cols >= 18 are uninitialized pad
    # (8400 = 127*66 + 18), zeroed via a DMA from vwpad's zero row (compute
    # engines cannot target a lone partition 127, and a memset+overlapping-DMA
    # combination wedges the HW even though CoreSim accepts it).
    P2 = tile1("P2", (128, NPP * NC_COL))
    nc.sync.dma_start(P2[127:128, 18 * NC_COL:],
                      d["vwpad"].ap()[0:1, 0:(NPP - 18) * NC_COL])
    nc.sync.dma_start(
        P2[0:127, :],
        d["pred"].ap()[0:127 * NPP, :].rearrange("(p n) c -> p (n c)", n=NPP))
    nc.sync.dma_start(
        P2[127:128, 0:18 * NC_COL],
        d["pred"].ap()[127 * NPP:NANCH, :].rearrange("(p n) c -> p (n c)", p=1))

    P3 = P2[:, :].rearrange("p (n c) -> p n c", c=NC_COL)   # [128, 66, 37]

    sg = tile1("sg", (128, NPP))
    act(sg[:, :], P3[:, :, 4], AF.Sigmoid)
    s2 = tile1("s2", (128, NPP))
    ts(s2[:, :], sg[:, :], -0.5, 0.0, OP.add, OP.max)       # relu(sig-0.5)
    ts(s2[:, :], s2[:, :], 0.001, None, OP.add)

    mk = tile1("mk", (128, NPP))
    nc.vector.tensor_reduce(mk[:, :], P3[:, :, 5:NC_COL], AX.X, OP.add,
                            apply_absolute_value=True)

    gm1 = tile1("gm1")
    nc.vector.tensor_reduce(gm1[:, :], P3[:, :, 0:4], AX.XY, OP.max)
    gmax = allmax_col("gmax", gm1[:, :])
    fsc = tile1("fsc")
    ts(fsc[:, :], gmax[:, :], 1.2, 639.0, OP.is_le, OP.mult)
    ts(fsc[:, :], fsc[:, :], 1.0, None, OP.add)

    dxa = tile1("dxa", (128, NPP))
    dya = tile1("dya", (128, NPP))
    act(dxa[:, :], P3[:, :, 0], AF.Abs, bias=-320.0, scale=fsc[:, :])
    act(dya[:, :], P3[:, :, 1], AF.Abs, bias=-320.0, scale=fsc[:, :])
    uxy = tile1("uxy", (128, NPP))
    tt(uxy[:, :], dxa[:, :], dya[:, :], OP.add)
    cwf = tile1("cwf", (128, NPP))
    ts(cwf[:, :], uxy[:, :], -1.0 / 640.0, 1.0, OP.mult, OP.add)
    ts(cwf[:, :], cwf[:, :], 0.0, 0.5, OP.max, OP.mult)
    ts(cwf[:, :], cwf[:, :], 0.5, None, OP.add)

    score = tile1("score", (128, NPP))
    tt(score[:, :], s2[:, :], mk[:, :], OP.mult)
    tt(score[:, :], score[:, :], cwf[:, :], OP.mult)

    vmax8 = tile1("vmax8", (128, 8))
    vidx8 = tile1("vidx8", (128, 8), U32)
    nc.vector.max_with_indices(vmax8[:, :], vidx8[:, :], score[:, :])

    gsc = allmax_col("gsc", vmax8[:, 0:1])

    af = tile1("af")
    nc.vector.tensor_copy(af[:, :], vidx8[:, 0:1])
    ts(af[:, :], af[:, :], pio66[:, :], -BIG, OP.add, OP.add)
    wm = tile1("wm")
    ts(wm[:, :], vmax8[:, 0:1], gsc[:, :], None, OP.is_ge)
    cand = tile1("cand")
    tt(cand[:, :], af[:, :], wm[:, :], OP.mult)
    ts(cand[:, :], cand[:, :], BIG, -1.0, OP.add, OP.mult)
    pmx = ps.tile([1, 128], F32, tag="ps", name=f"pmx{rep}")
    nc.tensor.transpose(pmx[:, :], cand[:, :], i128[:, :])
    a_f = tile1("a_f", (1, 1))
    nc.vector.tensor_reduce(a_f[0:1, :], pmx[:, :], AX.X, OP.max)
    ts(a_f[0:1, :], a_f[0:1, :], -1.0, None, OP.mult)
    a_i = tile1("a_i", (1, 1), I32)
    nc.vector.tensor_copy(a_i[0:1, :], a_f[0:1, :])

    if stage <= 1:
        metas = tile1("metas", (1, 8))
        nc.vector.memset(metas[:, :], 0.0)
        nc.vector.tensor_copy(metas[0:1, 0:1], a_f[0:1, :])
        nc.sync.dma_start(d["meta"].ap(), metas[:, :])
        ctx.close()
        return

    # ---------------- stage G: gather winner row; box -> windows ----------
    row1 = tile1("row1", (1, NC_COL))
    with nc.gpsimd.register(f"aoff{rep}") as areg:
        nc.gpsimd.reg_load(areg, a_i[0:1, 0:1])
        aoff = nc.gpsimd.snap(areg, min_val=0, max_val=NANCH - 1)
        nc.gpsimd.dma_start(row1[:, :], d["pred"].ap()[bass.ds(aoff, 1), :])

    psB = ps.tile([128, NC_COL], F32, tag="ps", name=f"psB{rep}")
    nc.tensor.matmul(psB[:, :], ones1[:, :], row1[:, :], start=True, stop=True)
    bc37 = tile1("bc37", (128, NC_COL))
    nc.scalar.copy(bc37[:, :], psB[:, :])

    psT = ps.tile([32, 1], F32, tag="ps", name=f"psT{rep}")
    nc.tensor.transpose(psT[:, :], row1[:, 5:NC_COL], id1[:, :])
    coefT = tile1("coefT", (32, 1))
    nc.scalar.copy(coefT[:, :], psT[:, :])

    # box -> fb (full-res rect bounds), [128,1] broadcast columns
    halfw = tile1("halfw")
    halfh = tile1("halfh")
    ts(halfw[:, :], bc37[:, 2:3], 0.5, None, OP.mult)
    ts(halfh[:, :], bc37[:, 3:4], 0.5, None, OP.mult)

    def clipped(dst, src_col, half, op, sxy):
        t = tile1(dst + "_t")
        tt(t[:, :], bc37[:, src_col:src_col + 1], half[:, :], op)
        ts(t[:, :], t[:, :], 0.0, float(IMGSZ - 1), OP.max, OP.min)
        o = tile1(dst)
        ts(o[:, :], t[:, :], sxy, None, OP.mult)
        return o

    SX, SY = W0 / IMGSZ, H0 / IMGSZ
    fb0 = clipped("fb0", 0, halfw, OP.subtract, SX)
    fb1 = clipped("fb1", 1, halfh, OP.subtract, SY)
    fb2 = clipped("fb2", 0, halfw, OP.add, SX)
    fb3 = clipped("fb3", 1, halfh, OP.add, SY)

    # column window: m = clamp(round(fb0/6 - 1.5), 0, 554); c0 = 6m
    c0m = tile1("c0m")
    ts(c0m[:, :], fb0[:, :], 1.0 / 6.0, 1.5, OP.mult, OP.subtract)
    ts(c0m[:, :], c0m[:, :], 0.0, 554.0, OP.max, OP.min)
    m_i = tile1("m_i", (1, 1), I32)
    nc.vector.tensor_copy(m_i[0:1, :], c0m[0:1, :])
    c0_i = tile1("c0_i", (1, 1), I32)
    ts(c0_i[:, :], m_i[:, :], 6, None, OP.mult)
    c0f1 = tile1("c0f1", (1, 1))
    nc.vector.tensor_copy(c0f1[0:1, :], c0_i[0:1, :])
    c0col = bcast_col("c0col", c0f1[:, :])

    # m160 column window: ww = clamp(floor(m/4) - 1, 0, 132) via
    # round(m/4 - 1.375) (fractions of m/4 are k/4 so the .375 offset rounds
    # to exactly floor(m/4) - 1)
    m_f = tile1("m_f", (1, 1))
    nc.vector.tensor_copy(m_f[0:1, :], m_i[0:1, :])
    wwf = tile1("wwf", (1, 1))
    ts(wwf[:, :], m_f[:, :], 0.25, 1.375, OP.mult, OP.subtract)
    ts(wwf[:, :], wwf[:, :], 0.0, float(160 - WW160), OP.max, OP.min)
    ww_i = tile1("ww_i", (1, 1), I32)
    nc.vector.tensor_copy(ww_i[0:1, :], wwf[0:1, :])

    # row window: rw = clamp(round(fb1 - r0 - 1.5), 0, 142)
    rwt = tile1("rwt", (1, 1))
    tt(rwt[0:1, :], fb1[0:1, :], r0c[:, :], OP.subtract)
    ts(rwt[:, :], rwt[:, :], 1.5, None, OP.subtract)
    ts(rwt[:, :], rwt[:, :], 0.0, float(ROWS - RWIN), OP.max, OP.min)
    rw_i = tile1("rw_i", (1, 1), I32)
    nc.vector.tensor_copy(rw_i[0:1, :], rwt[0:1, :])
    rw_f = tile1("rw_f", (1, 1))
    nc.vector.tensor_copy(rw_f[0:1, :], rw_i[0:1, :])
    rbase1 = tile1("rbase1", (1, 1))
    tt(rbase1[0:1, :], rw_f[0:1, :], r0c[:, :], OP.add)
    rbase = bcast_col("rbase", rbase1[:, :])
    riog = tile1("riog")                       # global row index per partition
    tt(riog[:, :], pio1[:, :], rbase[:, :], OP.add)

    # meta output for the host coverage check: [a, fb0..3, c0, rw, fsc]
    metas = tile1("metas", (1, 8))
    nc.vector.tensor_copy(metas[0:1, 0:1], a_f[0:1, :])
    nc.vector.tensor_copy(metas[0:1, 1:2], fb0[0:1, :])
    nc.vector.tensor_copy(metas[0:1, 2:3], fb1[0:1, :])
    nc.vector.tensor_copy(metas[0:1, 3:4], fb2[0:1, :])
    nc.vector.tensor_copy(metas[0:1, 4:5], fb3[0:1, :])
    nc.vector.tensor_copy(metas[0:1, 5:6], c0f1[0:1, :])
    nc.vector.tensor_copy(metas[0:1, 6:7], rw_f[0:1, :])
    nc.vector.tensor_copy(metas[0:1, 7:8], fsc[0:1, :])
    nc.sync.dma_start(d["meta"].ap(), metas[:, :])

    if stage <= 2:
        ctx.close()
        return

    # ---------------- stage M: windowed mask pipeline ----------------
    ahst = tile1("ahst", (MROWS, SROWS))
    nc.sync.dma_start(ahst[:, :], d["ahst"].ap())

    r82w = tile1("r82w", (SROWS, RWIN))
    protosw = tile1("protosw", (32, MROWS * WW160))
    awW = tile1("awW", (WW160, SWIN))
    vww = tile1("vww", (SWIN, WWIN))
    xw = tile1("xw", (128, 3 * WWIN))
    with nc.gpsimd.register(f"mo{rep}") as mreg, \
            nc.gpsimd.register(f"wo{rep}") as wreg, \
            nc.gpsimd.register(f"co_{rep}") as creg, \
            nc.gpsimd.register(f"ro{rep}") as rreg:
        nc.gpsimd.reg_load(mreg, m_i[0:1, 0:1])
        nc.gpsimd.reg_load(wreg, ww_i[0:1, 0:1])
        nc.gpsimd.reg_load(creg, c0_i[0:1, 0:1])
        nc.gpsimd.reg_load(rreg, rw_i[0:1, 0:1])
        mo = nc.gpsimd.snap(mreg, min_val=0, max_val=554)
        wo = nc.gpsimd.snap(wreg, min_val=0, max_val=160 - WW160)
        co = nc.gpsimd.snap(creg, min_val=0, max_val=W0 - WWIN)
        ro = nc.gpsimd.snap(rreg, min_val=0, max_val=ROWS - RWIN)
        nc.gpsimd.dma_start(
            protosw[:, :].rearrange("c (h w) -> c h w", w=WW160),
            d["protos"].ap().rearrange("c (h w) -> c h w", w=160)
            [:, :, bass.ds(wo, WW160)])
        nc.gpsimd.dma_start(awW[:, :],
                            d["awtp"].ap()[bass.ds(wo, WW160), bass.ds(mo, SWIN)])
        nc.gpsimd.dma_start(vww[:, :],
                            d["vwpad"].ap()[bass.ds(mo, SWIN), bass.ds(co, WWIN)])
        nc.gpsimd.dma_start(r82w[:, :], d["r82"].ap()[:, bass.ds(ro, RWIN)])
        xt = d["xs"].ap().transpose([1, 0, 2])      # [270, 3, 3840]
        nc.gpsimd.dma_start(
            xw[:, :].rearrange("p (c w) -> p c w", c=3),
            xt[bass.ds(ro, RWIN), :, bass.ds(co, WWIN)])

    # coef matvec over the window: m160w[1, (h24, w28)]
    psM = ps.tile([1, MROWS * WW160], F32, tag="psM", name=f"psM{rep}", bufs=1)
    nc.tensor.matmul(psM[0:1, 0:512], coefT, protosw[:, 0:512],
                     start=True, stop=True)
    nc.tensor.matmul(psM[0:1, 512:MROWS * WW160], coefT,
                     protosw[:, 512:MROWS * WW160], start=True, stop=True)
    m160wf = tile1("m160wf", (1, MROWS * WW160))
    nc.scalar.copy(m160wf[:, :], psM[:, :])
    m160r = tile1("m160r", (MROWS, WW160))
    nc.sync.dma_start(
        m160r[:, :],
        m160wf[:, :].rearrange("q (h w) -> (q h) w", w=WW160))

    # step1: P1T[w28, j82] = sum_h m160r[h, w] * ahst[h, j]
    psP = ps.tile([WW160, SROWS], F32, tag="ps", name=f"psP{rep}")
    nc.tensor.matmul(psP[:, :], m160r[:, :], ahst[:, :], start=True, stop=True)
    p1 = tile1("p1", (WW160, SROWS))
    nc.scalar.copy(p1[:, :], psP[:, :])

    # step2: m640win[j82, i88] = sum_w P1T[w, j] * awW[w, i]
    psQ = ps.tile([SROWS, SWIN], F32, tag="ps", name=f"psQ{rep}")
    nc.tensor.matmul(psQ[:, :], p1[:, :], awW[:, :], start=True, stop=True)
    s_win = tile1("s_win", (SROWS, SWIN))
    act(s_win[:, :], psQ[:, :], AF.Sigmoid)

    # uTw[i88, r128] = sum_p s_win[p, i] * r82w[p, r]  (row-windowed directly)
    psU = ps.tile([SWIN, RWIN], F32, tag="ps", name=f"psU{rep}")
    nc.tensor.matmul(psU[:, :], s_win[:, :], r82w[:, :], start=True, stop=True)
    uTw = tile1("uTw", (SWIN, RWIN))
    nc.scalar.copy(uTw[:, :], psU[:, :])

    if stage <= 3:
        ctx.close()
        return

    # ---------------- stage O: threshold + rect + multiply ----------------
    xcol = tile1("xcol", (128, WWIN))
    xiota = tile1("xiota", (128, WWIN))
    nc.sync.dma_start(xiota[:, :], d["xiota"].ap())
    ts(xcol[:, :], xiota[:, :], c0col[:, :], None, OP.add)
    cma = tile1("cma", (128, WWIN))
    ts(cma[:, :], xcol[:, :], fb0[:, :], 255.0, OP.is_ge, OP.mult)
    cmb = tile1("cmb", (128, WWIN))
    ts(cmb[:, :], xcol[:, :], fb2[:, :], None, OP.is_lt)
    cm255 = tile1("cm255", (128, WWIN))
    tt(cm255[:, :], cma[:, :], cmb[:, :], OP.mult)

    psW = ps.tile([RWIN, WWIN], F32, tag="psW", name=f"psW{rep}", bufs=1)
    nc.tensor.matmul(psW[:, 0:512], uTw[:, :], vww[:, 0:512],
                     start=True, stop=True)
    nc.tensor.matmul(psW[:, 512:WWIN], uTw[:, :], vww[:, 512:WWIN],
                     start=True, stop=True)
    sgn = tile1("sgn", (RWIN, WWIN))
    act(sgn[:, :], psW[:, :], AF.Sign, bias=-MASK_THR)

    rm = tile1("rm")
    rmb = tile1("rmb")
    ts(rm[:, :], riog[:, :], fb1[:, :], None, OP.is_ge)
    ts(rmb[:, :], riog[:, :], fb3[:, :], None, OP.is_lt)
    tt(rm[:, :], rm[:, :], rmb[:, :], OP.mult)
    bm = tile1("bm", (RWIN, WWIN))
    ts(bm[:, :], sgn[:, :], 0.0, rm[:, :], OP.max, OP.mult)
    bm3 = tile1("bm3", (RWIN, WWIN))
    tt(bm3[:, :], bm[:, :], cm255[:, :], OP.mult)

    res = tile1("res", (RWIN, 3 * WWIN))
    for ch in range(3):
        tt(res[:, WWIN * ch:WWIN * (ch + 1)],
           xw[:, WWIN * ch:WWIN * (ch + 1)], bm3[:, :], OP.mult)

    out_t = d["out"].ap().transpose([1, 0, 2])          # [270, 3, 3840]
    with nc.gpsimd.register(f"co2{rep}") as creg, \
            nc.gpsimd.register(f"ro3{rep}") as rreg:
        nc.gpsimd.reg_load(creg, c0_i[0:1, 0:1])
        nc.gpsimd.reg_load(rreg, rw_i[0:1, 0:1])
        co = nc.gpsimd.snap(creg, min_val=0, max_val=W0 - WWIN)
        ro = nc.gpsimd.snap(rreg, min_val=0, max_val=ROWS - RWIN)
        nc.gpsimd.dma_start(
            out_t[bass.ds(ro, RWIN), :, bass.ds(co, WWIN)],
            res[:, :].rearrange("p (c w) -> p c w", c=3))

    ctx.close()


# ---------------------------------------------------------------------------
# host orchestration
# ---------------------------------------------------------------------------

_NC_CACHE = None


def _get_nc():
    global _NC_CACHE
    if _NC_CACHE is None:
        _NC_CACHE = _build_nc()
    return _NC_CACHE


def _make_in_maps(x_raw, pred2, proto2, shared, percore):
    in_maps = []
    for c in range(N_CORES):
        pc = percore[c]
        ha = pc["ha"]
        in_maps.append({
            "pred": pred2,
            "xs": np.ascontiguousarray(x_raw[0, :, ROWS * c:ROWS * (c + 1), :]),
            "protos": np.ascontiguousarray(
                proto2[:, ha:ha + MROWS, :].reshape(32, MROWS * 160)),
            "ahst": pc["ahst"],
            "awtp": shared["awtp"],
            "r82": pc["r82"],
            "vwpad": shared["vwpad"],
            "ones1": shared["ones1"],
            "id1": shared["id1"],
            "i128": shared["i128"],
            "xiota": shared["xiota"],
            "pio1": shared["pio1"],
            "pio66": shared["pio66"],
            "r0c": pc["r0c"],
        })
    return in_maps


def _numpy_fallback(x_raw, pred, proto):
    """Exact slow-path reference (only used if the rect exceeds the device
    windows, which cannot happen for in-distribution inputs)."""
    p = pred[0]
    boxes, cls, coef = p[:, :4], p[:, 4], p[:, 5:]
    s1 = np.maximum(1.0 / (1.0 + np.exp(-cls)) - 0.5, 0) + np.float32(0.001)
    mk = np.abs(coef).sum(-1)
    f = np.float32(640.0 if boxes.max() <= 1.2 else 1.0)
    dxdy = np.abs(boxes[:, :2] * f - 320.0) / 320.0
    cw = np.maximum(1.0 - 0.5 * (dxdy[:, 0] + dxdy[:, 1]), 0.0)
    a = int(np.argmax(s1 * mk * (0.5 + 0.5 * cw)))
    fcoef = coef[a]
    cx, cy, w, h = boxes[a]
    xyxy = np.clip(np.array([cx - w / 2, cy - h / 2, cx + w / 2, cy + h / 2],
                            np.float32), 0.0, IMGSZ - 1)
    fb = xyxy * np.array([W0 / IMGSZ, H0 / IMGSZ, W0 / IMGSZ, H0 / IMGSZ],
                         np.float32)
    Ah = _weight_mat(160, IMGSZ)
    Aw = _weight_mat(160, IMGSZ)
    Vh = _weight_mat(IMGSZ, H0)
    Vw = _weight_mat(IMGSZ, W0)
    m160 = (fcoef @ proto[0].reshape(32, -1)).reshape(160, 160)
    m640 = Ah.T @ m160 @ Aw
    s640 = 1.0 / (1.0 + np.exp(-m640))
    m_orig = (Vh.T @ s640 @ Vw).astype(np.float32)
    ys = np.arange(H0, dtype=np.float32)[:, None]
    xs = np.arange(W0, dtype=np.float32)[None, :]
    rect = (xs >= fb[0]) & (xs < fb[2]) & (ys >= fb[1]) & (ys < fb[3])
    bm = ((m_orig > MASK_THR) & rect).astype(np.float32)
    return (np.clip(x_raw * 255.0, 0.0, 255.0) * bm[None, None]).astype(np.float32)


def _covered(metas):
    """Check every rect pixel lies inside each core's written window."""
    fb0, fb1, fb2, fb3 = metas[0][1], metas[0][2], metas[0][3], metas[0][4]
    if fb2 <= fb0 or fb3 <= fb1:
        return True
    c0 = metas[0][5]
    cols = np.arange(W0, dtype=np.float32)
    csel = (cols >= fb0) & (cols < fb2)
    if csel.any():
        lo, hi = np.where(csel)[0][[0, -1]]
        if not (c0 <= lo and hi < c0 + WWIN):
            return False
    rows = np.arange(H0, dtype=np.float32)
    rsel = (rows >= fb1) & (rows < fb3)
    for c in range(N_CORES):
        sel = rsel[ROWS * c:ROWS * (c + 1)]
        if sel.any():
            rw = metas[c][6]
            lo, hi = np.where(sel)[0][[0, -1]]
            if not (rw <= lo and hi < rw + RWIN):
                return False
    return True


def kernel(x_raw, pred, proto):
    x_raw = np.ascontiguousarray(np.asarray(x_raw, dtype=np.float32))
    pred = np.ascontiguousarray(np.asarray(pred, dtype=np.float32))
    proto = np.ascontiguousarray(np.asarray(proto, dtype=np.float32))

    nc = _get_nc()
    shared, percore = _host_consts()
    pred2 = np.ascontiguousarray(pred[0])
    proto2 = proto[0]
    in_maps = _make_in_maps(x_raw, pred2, proto2, shared, percore)

    res = bass_utils.run_bass_kernel_spmd(nc, in_maps,
                                          core_ids=list(range(N_CORES)))

    metas = [res.results[c]["meta"][0] for c in range(N_CORES)]
    if not _covered(metas):
        return _numpy_fallback(x_raw, pred, proto)

    out = np.concatenate([res.results[c]["out"] for c in range(N_CORES)],
                         axis=1)          # [3, 2160, 3840]
    return out[None]


if __name__ == "__main__":
    import jax
    with jax.default_device(jax.devices("cpu")[0]):
        import reference as R
        inputs = R.setup_inputs()
        inputs = {k: np.asarray(v) for k, v in inputs.items()}
    out = kernel(**inputs)
    ref = np.load("/tmp/ref_out.npy")
    print("absmax:", np.abs(out - ref).max())



# revision 12
# speedup vs baseline: 2.7920x; 2.7920x over previous
"""Trainium2 Bass kernel for nn_End2EndTongueROI_Dynamic_NMS.

Key algebraic facts used (verified against the reference):
  - Greedy NMS always keeps the top-scored box first and fi=argmax(keep)=0,
    so the whole top-k/NMS tail reduces to argmax(score) over 8400 anchors.
  - score's /max(maskness) normalization and /32 mean are positive scalings
    shared by all anchors -> argmax-invariant -> dropped on device.
  - The rect is built from the *unscaled* xyxy box (reference quirk), and the
    reference's pred is U[0,1), so xyxy < 1.5 and the full-res rect lives in
    rows [0, 5.1) x cols [0, 9).  The device therefore computes a tiny fully
    STATIC window (rows 0:16 x cols 0:64 of the full-res image, owned by
    core 0); everything outside is exactly 0 in the reference output.  A host
    coverage check on the device-reported box falls back to exact numpy if
    the rect ever exceeds the window (impossible for in-distribution inputs).
  - Both resizes are linear with exact jax f32 weight matrices; the window's
    dependency cone is rows/cols 0:4 of the 160x160 proto plane, 0:8 x 0:12
    of the 640 plane.  The w-resize leg (proto @ Aw) is folded on the host
    (coef-independent), and the coef contraction + h-resize run as ONE PE
    matmul via a (coef x Ah)-tiled [128, x] layout (32 coefs x 4 proto rows
    = 128 partitions), so no on-device reshape DMAs are needed.
  - All constants that the old kernel DMA'd (identity-128, iotas, one-hot
    matrices) are generated on device with iota/memset/affine ops; pred is
    zero-padded to 8448 rows on host so the score stage is a single DMA.

Sharding: the problem is latency-bound (one tiny box); all 8 cores run the
identical replicated program (no collectives), core 0's output is used.
"""
import numpy as np

import concourse.bacc as bacc
import concourse.bass as bass
import concourse.mybir as mybir
import concourse.tile as tile
from concourse import bass_isa, bass_utils

F32 = mybir.dt.float32
I32 = mybir.dt.int32
U32 = mybir.dt.uint32

N_CORES = 8
H0, W0 = 2160, 3840
IMGSZ = 640
MASK_THR = 0.72
NANCH, NC_COL = 8400, 37
NPP = 66                      # anchors per partition (128*66 = 8448)
NPAD = 128 * NPP
ROWS = H0 // N_CORES          # 270 rows per core

RWIN = 16                     # output row window (global rows 0:16, core 0)
WWIN = 64                     # output col window
SROWS = 8                     # s640 row window
SWIN = 12                     # s640 col window
MH = 4                        # m160 row window (32*4 = 128 partitions)
MW = 4                        # m160 col window
MKSPL = 38                    # maskness cols on DVE (rest on gpsimd)
# sentinel for the argmin-over-winners trick; power of two > NANCH so that
# af - BIG and +BIG round-trip exactly in f32
BIG = 16384.0


# ---------------------------------------------------------------------------
# host-side resize weights (exact replica of jax.image.resize bilinear)
# ---------------------------------------------------------------------------

def _weight_mat(in_size, out_size):
    dt = np.float32
    scale = dt(out_size / in_size)
    inv_scale = dt(1.0) / scale
    sample_f = (np.arange(out_size, dtype=dt) + dt(0.5)) * inv_scale - dt(0.5)
    x = np.abs(sample_f[None, :] - np.arange(in_size, dtype=dt)[:, None])
    w = np.maximum(dt(0), dt(1) - x).astype(dt)
    tot = w.sum(axis=0, keepdims=True).astype(dt)
    w = np.where(np.abs(tot) > 1000.0 * np.finfo(np.float32).eps,
                 w / np.where(tot != 0, tot, 1), 0).astype(dt)
    ok = (sample_f >= -0.5) & (sample_f <= in_size - 0.5)
    return np.where(ok[None, :], w, 0).astype(dt)


_CONST_CACHE = None


def _host_consts():
    """Static constant tensors. Returns dict; per-core pieces are lists."""
    global _CONST_CACHE
    if _CONST_CACHE is not None:
        return _CONST_CACHE
    Ah = _weight_mat(160, IMGSZ)      # [160, 640] (same for both axes)
    Vh = _weight_mat(IMGSZ, H0)       # [640, 2160]
    Vw = _weight_mat(IMGSZ, W0)       # [640, 3840]

    # window dependency-cone guarantees (all exact zeros by construction)
    assert (Ah[MH:, :SROWS] == 0).all()
    assert (Ah[MW:, :SWIN] == 0).all()
    assert (Vh[SROWS:, :RWIN] == 0).all()
    assert (Vw[SWIN:, :WWIN] == 0).all()

    ahst_tiled = np.tile(Ah[:MH, :SROWS], (32, 1)).astype(np.float32)  # [128,8]
    awin = np.ascontiguousarray(Ah[:MW, :SWIN])                        # [4,12]
    vww = np.ascontiguousarray(Vw[:SWIN, :WWIN])                       # [12,64]
    vhw = []
    for c in range(N_CORES):
        r0 = ROWS * c
        vhw.append(np.ascontiguousarray(Vh[:SROWS, r0:r0 + RWIN]))     # [8,16]
    _CONST_CACHE = dict(Ah=Ah, Vh=Vh, Vw=Vw, ahst_tiled=ahst_tiled,
                        awin=awin, vww=vww, vhw=vhw)
    return _CONST_CACHE


# ---------------------------------------------------------------------------
# device program (identical for all cores; per-core data comes via inputs)
# ---------------------------------------------------------------------------

def _build_nc(stage=99, reps=1, loop_n=0):
    nc = bacc.Bacc("TRN2", target_bir_lowering=False, debug=False,
                   enable_asserts=False, num_devices=N_CORES)

    d = {}
    d["pred"] = nc.dram_tensor("pred", [NPAD, NC_COL], F32, kind="ExternalInput")
    d["cpk"] = nc.dram_tensor("cpk", [128, 24], F32, kind="ExternalInput")
    d["vws"] = nc.dram_tensor("vws", [SWIN, WWIN + RWIN], F32, kind="ExternalInput")
    d["xs"] = nc.dram_tensor("xs", [RWIN, 3 * WWIN], F32, kind="ExternalInput")

    d["out"] = nc.dram_tensor("out", [RWIN, 3 * WWIN], F32, kind="ExternalOutput")
    d["meta"] = nc.dram_tensor("meta", [1, 8], F32, kind="ExternalOutput")

    with tile.TileContext(nc) as tc:
        if loop_n:
            with tc.For_i(0, loop_n, 1):
                _program(nc, tc, d, stage, 0)
        else:
            for rep in range(reps):
                _program(nc, tc, d, stage, rep)
    nc.compile()
    return nc


def _program(nc, tc, d, stage=99, rep=0):
    AF = mybir.ActivationFunctionType
    OP = mybir.AluOpType
    AX = mybir.AxisListType
    import contextlib
    ctx = contextlib.ExitStack()

    sb = ctx.enter_context(tc.tile_pool(name="sb", bufs=1))
    ps = ctx.enter_context(tc.tile_pool(name="ps", bufs=2,
                                        space=bass.MemorySpace.PSUM))

    _bias_cache = {}

    def cbias(val):
        if val not in _bias_cache:
            t = sb.tile([128, 1], F32, tag=f"cb{rep}_{len(_bias_cache)}",
                        name=f"cb{rep}_{len(_bias_cache)}")
            nc.gpsimd.memset(t[:, :], val)
            _bias_cache[val] = t
        return _bias_cache[val]

    def act(out_ap, in_ap, func, bias=0.0, scale=1.0):
        nparts = in_ap.shape[0]
        nc.scalar.activation(out_ap, in_ap, func,
                             bias=cbias(float(bias))[0:nparts, :],
                             scale=scale)

    def ts(eng, out_ap, in_ap, s1, s2, op0, op1=None):
        eng.tensor_scalar(out_ap, in_ap, s1, s2, op0,
                          *([] if op1 is None else [op1]))

    def tile1(tag, shape=(128, 1), dtype=F32):
        return sb.tile(list(shape), dtype, tag=f"{tag}{rep}",
                       name=f"{tag}_{rep}")

    V, G = nc.vector, nc.gpsimd

    # ---------------- phase 0: input DMAs + device-built constants --------
    P2 = tile1("P2", (128, NPP * NC_COL))
    nc.sync.dma_start(
        P2[:, :],
        d["pred"].ap().rearrange("(p n) c -> p (n c)", n=NPP))

    cpk = tile1("cpk", (128, 24))
    nc.scalar.dma_start(cpk[:, :], d["cpk"].ap())
    vws = tile1("vws", (SWIN, WWIN + RWIN))
    nc.scalar.dma_start(vws[:, :], d["vws"].ap())
    xst = tile1("xst", (RWIN, 3 * WWIN))
    nc.scalar.dma_start(xst[:, :], d["xs"].ap())
    # cpk layout: col 0 riog(=270c+p); cols 2:10 ahst_tiled; cols 10:22 protoAW
    riog = cpk[:, 0:1]
    ahst = cpk[:, 2:10]
    protoAW = cpk[:, 10:22]

    xio_i = tile1("xio_i", (128, 128), I32)
    G.iota(xio_i[:, :], pattern=[[1, 128]], base=0, channel_multiplier=0)
    xio = tile1("xio", (128, 128))
    G.tensor_copy(xio[:, :], xio_i[:, :])
    pio_i = tile1("pio_i", (128, 1), I32)
    G.iota(pio_i[:, :], pattern=[[1, 1]], base=0, channel_multiplier=1)
    pio = tile1("pio")
    G.tensor_copy(pio[:, :], pio_i[:, :])
    i128 = tile1("i128", (128, 128))
    ts(G, i128[:, :], xio[:, :], pio[:, 0:1], None, OP.is_equal)
    pio66 = tile1("pio66")
    ts(G, pio66[:, :], pio[:, :], 66.0, None, OP.mult)
    ones1 = tile1("ones1", (1, 128))
    G.memset(ones1[:, :], 1.0)
    # EMAT[c, p] = 1 iff p//4 == c  (for coef -> 128-partition spread)
    p4 = tile1("p4", (32, 1))
    ts(G, p4[:, :], pio[0:32, :], 4.0, None, OP.mult)
    p44 = tile1("p44", (32, 1))
    ts(G, p44[:, :], p4[:, :], 4.0, None, OP.add)
    e1 = tile1("e1", (32, 128))
    ts(G, e1[:, :], xio[0:32, :], p4[:, 0:1], None, OP.is_ge)
    em = tile1("em", (32, 128))
    ts(G, em[:, :], xio[0:32, :], p44[:, 0:1], None, OP.is_lt)
    G.tensor_tensor(em[:, :], em[:, :], e1[:, :], OP.mult)
    metas = tile1("metas", (1, 8))
    G.memset(metas[:, :], 0.0)

    # ---------------- stage S: score fusion + argmax ----------------
    P3 = P2[:, :].rearrange("p (n c) -> p n c", c=NC_COL)   # [128, 66, 37]

    sg = tile1("sg", (128, NPP))
    act(sg[:, :], P3[:, :, 4], AF.Sigmoid)
    s2 = tile1("s2", (128, NPP))
    ts(G, s2[:, :], sg[:, :], -0.5, 0.0, OP.add, OP.max)    # relu(sig-0.5)
    ts(G, s2[:, :], s2[:, :], 0.001, None, OP.add)

    # staging tile for one transpose: cols 0:8 top8, col 8 af, col 9 boxmax
    stg = tile1("stg", (128, 10))
    V.tensor_reduce(stg[:, 9:10], P3[:, :, 0:4], AX.XY, OP.max)
    mk = tile1("mk", (128, NPP))
    V.tensor_reduce(mk[:, :], P3[:, :, 5:NC_COL], AX.X, OP.add,
                    apply_absolute_value=True)

    # center weighting (assumes normalized boxes; host checks gmax <= 1.2)
    dxa = tile1("dxa", (128, NPP))
    dya = tile1("dya", (128, NPP))
    act(dxa[:, :], P3[:, :, 0], AF.Abs, bias=-320.0, scale=640.0)
    act(dya[:, :], P3[:, :, 1], AF.Abs, bias=-320.0, scale=640.0)
    uxy = tile1("uxy", (128, NPP))
    V.tensor_tensor(uxy[:, :], dxa[:, :], dya[:, :], OP.add)
    cwf = tile1("cwf", (128, NPP))
    ts(G, cwf[:, :], uxy[:, :], -1.0 / 640.0, 1.0, OP.mult, OP.add)
    ts(G, cwf[:, :], cwf[:, :], 0.0, 0.5, OP.max, OP.mult)
    ts(G, cwf[:, :], cwf[:, :], 0.5, None, OP.add)

    score = tile1("score", (128, NPP))
    V.tensor_tensor(score[:, :], s2[:, :], mk[:, :], OP.mult)
    V.tensor_tensor(score[:, :], score[:, :], cwf[:, :], OP.mult)

    vidx8 = tile1("vidx8", (128, 8), U32)
    V.max_with_indices(stg[:, 0:8], vidx8[:, :], score[:, :])
    aff = tile1("aff")
    V.tensor_copy(aff[:, :], vidx8[:, 0:1])
    ts(V, stg[:, 8:9], aff[:, :], pio66[:, 0:1], -BIG, OP.add, OP.add)

    pmax = ps.tile([1, 128], F32, tag="ps", name=f"pmax{rep}")
    nc.tensor.transpose(pmax[:, :], stg[:, 0:1], i128[:, :])
    paf = ps.tile([1, 128], F32, tag="ps", name=f"paf{rep}")
    nc.tensor.transpose(paf[:, :], stg[:, 8:9], i128[:, :])
    pgm = ps.tile([1, 128], F32, tag="ps", name=f"pgm{rep}")
    nc.tensor.transpose(pgm[:, :], stg[:, 9:10], i128[:, :])

    gsc = tile1("gsc", (1, 1))
    V.tensor_reduce(gsc[0:1, :], pmax[0:1, :], AX.X, OP.max)
    wm1 = tile1("wm1", (1, 128))
    ts(V, wm1[0:1, :], pmax[0:1, :], gsc[0:1, 0:1], None, OP.is_ge)
    cand = tile1("cand", (1, 128))
    V.tensor_tensor(cand[0:1, :], paf[0:1, :], wm1[0:1, :], OP.mult)
    ts(V, cand[0:1, :], cand[0:1, :], BIG, -1.0, OP.add, OP.mult)
    a_f = tile1("a_f", (1, 1))
    V.tensor_reduce(a_f[0:1, :], cand[0:1, :], AX.X, OP.max)
    ts(V, a_f[0:1, :], a_f[0:1, :], -1.0, None, OP.mult)
    a_i = tile1("a_i", (1, 1), I32)
    V.tensor_copy(a_i[0:1, :], a_f[0:1, :])
    gmax = tile1("gmax", (1, 1))
    V.tensor_reduce(gmax[0:1, :], pgm[0:1, :], AX.X, OP.max)

    if stage <= 1:
        V.tensor_copy(metas[0:1, 0:1], a_f[0:1, :])
        nc.scalar.dma_start(d["meta"].ap(), metas[:, :])
        ctx.close()
        return

    # ---------------- stage G: gather winner row ----------------
    row1 = tile1("row1", (1, NC_COL))
    rowT = tile1("rowT", (32, 1))
    with nc.gpsimd.register(f"aoff{rep}") as areg:
        nc.gpsimd.reg_load(areg, a_i[0:1, 0:1])
        aoff = nc.gpsimd.snap(areg, min_val=0, max_val=NANCH - 1)
        nc.gpsimd.dma_start(row1[:, :], d["pred"].ap()[bass.ds(aoff, 1), :])
        nc.gpsimd.dma_start(rowT[:, :],
                            d["pred"].ap()[bass.ds(aoff, 1), 5:NC_COL])

    # ---------------- stage M: windowed mask pipeline (PE/Act chain) ------
    psB = ps.tile([128, NC_COL], F32, tag="ps", name=f"psB{rep}")
    nc.tensor.matmul(psB[:, :], ones1[:, :], row1[:, :], start=True, stop=True)
    psE = ps.tile([128, 1], F32, tag="ps", name=f"psE{rep}")
    nc.tensor.matmul(psE[:, :], em[:, :], rowT[:, :], start=True, stop=True)
    coef128 = tile1("coef128")
    nc.scalar.copy(coef128[:, :], psE[:, :])
    SC = tile1("SC", (128, SROWS))
    ts(V, SC[:, :], ahst, coef128[:, 0:1], None, OP.mult)
    psQ = ps.tile([SROWS, SWIN], F32, tag="ps", name=f"psQ{rep}")
    nc.tensor.matmul(psQ[:, :], SC[:, :], protoAW, start=True, stop=True)
    s_win = tile1("s_win", (SROWS, SWIN))
    act(s_win[:, :], psQ[:, :], AF.Sigmoid)
    psU = ps.tile([SWIN, RWIN], F32, tag="ps", name=f"psU{rep}")
    nc.tensor.matmul(psU[:, :], s_win[:, :], vws[0:SROWS, WWIN:WWIN + RWIN],
                     start=True, stop=True)
    uTw = tile1("uTw", (SWIN, RWIN))
    nc.scalar.copy(uTw[:, :], psU[:, :])
    psW = ps.tile([RWIN, WWIN], F32, tag="ps", name=f"psW{rep}")
    nc.tensor.matmul(psW[:, :], uTw[:, :], vws[0:SWIN, 0:WWIN],
                     start=True, stop=True)
    sgn = tile1("sgn", (RWIN, WWIN))
    act(sgn[:, :], psW[:, :], AF.Sign, bias=-MASK_THR)

    # ---------------- stage R: rect masks (gpsimd, parallel with M) -------
    bc37 = tile1("bc37", (128, NC_COL))
    V.tensor_copy(bc37[:, :], psB[:, :])
    halfw = tile1("halfw")
    halfh = tile1("halfh")
    ts(G, halfw[:, :], bc37[:, 2:3], 0.5, None, OP.mult)
    ts(G, halfh[:, :], bc37[:, 3:4], 0.5, None, OP.mult)

    SX, SY = W0 / IMGSZ, H0 / IMGSZ

    def clipped(dst, src_col, half, op, sxy):
        t = tile1(dst + "_t")
        G.tensor_tensor(t[:, :], bc37[:, src_col:src_col + 1], half[:, :], op)
        ts(G, t[:, :], t[:, :], 0.0, float(IMGSZ - 1), OP.max, OP.min)
        o = tile1(dst)
        ts(G, o[:, :], t[:, :], sxy, None, OP.mult)
        return o

    fb0 = clipped("fb0", 0, halfw, OP.subtract, SX)
    fb1 = clipped("fb1", 1, halfh, OP.subtract, SY)
    fb2 = clipped("fb2", 0, halfw, OP.add, SX)
    fb3 = clipped("fb3", 1, halfh, OP.add, SY)

    cm255 = tile1("cm255", (RWIN, WWIN))
    cmb = tile1("cmb", (RWIN, WWIN))
    ts(G, cm255[:, :], xio[0:RWIN, 0:WWIN], fb0[0:RWIN, 0:1], 255.0,
       OP.is_ge, OP.mult)
    ts(G, cmb[:, :], xio[0:RWIN, 0:WWIN], fb2[0:RWIN, 0:1], None, OP.is_lt)
    G.tensor_tensor(cm255[:, :], cm255[:, :], cmb[:, :], OP.mult)
    rm = tile1("rm", (RWIN, 1))
    rmb = tile1("rmb", (RWIN, 1))
    ts(G, rm[:, :], riog[0:RWIN, :], fb1[0:RWIN, 0:1], None, OP.is_ge)
    ts(G, rmb[:, :], riog[0:RWIN, :], fb3[0:RWIN, 0:1], None, OP.is_lt)
    G.tensor_tensor(rm[:, :], rm[:, :], rmb[:, :], OP.mult)

    # meta output for the host coverage check: [a, fb0..3, gmax]
    G.tensor_copy(metas[0:1, 0:1], a_f[0:1, :])
    G.tensor_copy(metas[0:1, 1:2], fb0[0:1, :])
    G.tensor_copy(metas[0:1, 2:3], fb1[0:1, :])
    G.tensor_copy(metas[0:1, 3:4], fb2[0:1, :])
    G.tensor_copy(metas[0:1, 4:5], fb3[0:1, :])
    G.tensor_copy(metas[0:1, 5:6], gmax[0:1, :])
    nc.scalar.dma_start(d["meta"].ap(), metas[:, :])

    if stage <= 3:
        ctx.close()
        return

    # ---------------- stage O: threshold + rect + multiply ----------------
    bm = tile1("bm", (RWIN, WWIN))
    ts(V, bm[:, :], sgn[:, :], 0.0, rm[:, 0:1], OP.max, OP.mult)
    V.tensor_tensor(bm[:, :], bm[:, :], cm255[:, :], OP.mult)
    res = tile1("res", (RWIN, 3 * WWIN))
    for ch in range(3):
        V.tensor_tensor(res[:, WWIN * ch:WWIN * (ch + 1)],
                        xst[:, WWIN * ch:WWIN * (ch + 1)], bm[:, :], OP.mult)
    nc.sync.dma_start(d["out"].ap(), res[:, :])

    ctx.close()


# ---------------------------------------------------------------------------
# host orchestration
# ---------------------------------------------------------------------------

_NC_CACHE = None


def _get_nc():
    global _NC_CACHE
    if _NC_CACHE is None:
        _NC_CACHE = _build_nc()
    return _NC_CACHE


def _make_in_maps(x_raw, pred2, proto2, *_unused):
    hc = _host_consts()
    predp = np.zeros((NPAD, NC_COL), np.float32)
    predp[:NANCH] = pred2
    # protoAW[(c h), i] = sum_w proto[c, h, w] * Aw[w, i]  (w-resize folded)
    protoAW = np.einsum("chw,wi->chi",
                        proto2[:, :MH, :MW].astype(np.float32),
                        hc["awin"]).reshape(128, SWIN).astype(np.float32)
    in_maps = []
    for c in range(N_CORES):
        cpk = np.zeros((128, 24), np.float32)
        cpk[:, 0] = ROWS * c + np.arange(128, dtype=np.float32)
        cpk[:, 2:10] = hc["ahst_tiled"]
        cpk[:, 10:22] = protoAW
        vws = np.zeros((SWIN, WWIN + RWIN), np.float32)
        vws[:, :WWIN] = hc["vww"]
        vws[:SROWS, WWIN:] = hc["vhw"][c]
        xs = np.ascontiguousarray(
            x_raw[0, :, ROWS * c:ROWS * c + RWIN, 0:WWIN]
            .transpose(1, 0, 2).reshape(RWIN, 3 * WWIN))
        in_maps.append({"pred": predp, "cpk": cpk, "vws": vws, "xs": xs})
    return in_maps


def _numpy_fallback(x_raw, pred, proto):
    """Exact slow-path reference (only used if the rect exceeds the device
    windows, which cannot happen for in-distribution inputs)."""
    p = pred[0]
    boxes, cls, coef = p[:, :4], p[:, 4], p[:, 5:]
    s1 = np.maximum(1.0 / (1.0 + np.exp(-cls)) - 0.5, 0) + np.float32(0.001)
    mk = np.abs(coef).sum(-1)
    f = np.float32(640.0 if boxes.max() <= 1.2 else 1.0)
    dxdy = np.abs(boxes[:, :2] * f - 320.0) / 320.0
    cw = np.maximum(1.0 - 0.5 * (dxdy[:, 0] + dxdy[:, 1]), 0.0)
    a = int(np.argmax(s1 * mk * (0.5 + 0.5 * cw)))
    fcoef = coef[a]
    cx, cy, w, h = boxes[a]
    xyxy = np.clip(np.array([cx - w / 2, cy - h / 2, cx + w / 2, cy + h / 2],
                            np.float32), 0.0, IMGSZ - 1)
    fb = xyxy * np.array([W0 / IMGSZ, H0 / IMGSZ, W0 / IMGSZ, H0 / IMGSZ],
                         np.float32)
    Ah = _weight_mat(160, IMGSZ)
    Aw = _weight_mat(160, IMGSZ)
    Vh = _weight_mat(IMGSZ, H0)
    Vw = _weight_mat(IMGSZ, W0)
    m160 = (fcoef @ proto[0].reshape(32, -1)).reshape(160, 160)
    m640 = Ah.T @ m160 @ Aw
    s640 = 1.0 / (1.0 + np.exp(-m640))
    m_orig = (Vh.T @ s640 @ Vw).astype(np.float32)
    ys = np.arange(H0, dtype=np.float32)[:, None]
    xs = np.arange(W0, dtype=np.float32)[None, :]
    rect = (xs >= fb[0]) & (xs < fb[2]) & (ys >= fb[1]) & (ys < fb[3])
    bm = ((m_orig > MASK_THR) & rect).astype(np.float32)
    return (np.clip(x_raw * 255.0, 0.0, 255.0) * bm[None, None]).astype(np.float32)


def _covered(meta0):
    """Check the whole rect lies inside core 0's static window and the
    boxes were normalized (device assumes the x640 center scaling)."""
    _a, fb0, fb1, fb2, fb3, gmax = meta0[:6]
    if gmax > 1.2:
        return False
    if fb2 <= fb0 or fb3 <= fb1:
        return True
    return fb2 <= WWIN and fb3 <= RWIN


def kernel(x_raw, pred, proto):
    x_raw = np.ascontiguousarray(np.asarray(x_raw, dtype=np.float32))
    pred = np.ascontiguousarray(np.asarray(pred, dtype=np.float32))
    proto = np.ascontiguousarray(np.asarray(proto, dtype=np.float32))

    nc = _get_nc()
    in_maps = _make_in_maps(x_raw, pred[0], proto[0])

    res = bass_utils.run_bass_kernel_spmd(nc, in_maps,
                                          core_ids=list(range(N_CORES)))

    meta0 = res.results[0]["meta"][0]
    if not _covered(meta0):
        return _numpy_fallback(x_raw, pred, proto)

    out = np.zeros((1, 3, H0, W0), np.float32)
    win = res.results[0]["out"].reshape(RWIN, 3, WWIN).transpose(1, 0, 2)
    out[0, :, 0:RWIN, 0:WWIN] = win
    return out


if __name__ == "__main__":
    import jax
    with jax.default_device(jax.devices("cpu")[0]):
        import reference as R
        inputs = R.setup_inputs()
        inputs = {k: np.asarray(v) for k, v in inputs.items()}
    out = kernel(**inputs)
    ref = np.load("/tmp/ref_out.npy")
    print("absmax:", np.abs(out - ref).max())
